# revision 1
# baseline (speedup 1.0000x reference)
"""CopyGenerator kernel for 8x Trainium2 NeuronCores (Bass/Tile).

Computation (see reference):
    logits = hidden @ W.T + b            [BT, V]   (pad column masked to -inf)
    prob   = softmax(logits, axis=1)
    p_copy = sigmoid(hidden @ w_copy + b_copy)
    out    = concat([prob * (1 - p_copy),
                     einsum('bts,bsc', attn*p_copy, src_map)], axis=1)

Sharding: vocab dim of W/b/out_prob split 8 ways (tensor parallel).
All operand transposes are done on the host (free); the device only does
matmuls + exp + scaling.  W^T stays resident in SBUF (bf16), exp(logits)
stays in SBUF (never round-trips to DRAM), and the per-token normalizer
is AllReduced across cores in groups of token tiles, pipelined behind
the next group's matmuls.  Matmuls use 1024-wide moving operands into
double-bank PSUM tiles to amortize per-instruction overhead.  The last
two groups are single tiles so the final (latency-bound, ~35us)
AllReduce has minimal work behind it; the copy branch is emitted at the
end to fill that AllReduce's shadow.
"""

import sys

for _p in ("/opt/trn_rl_repo", "/root/.axon_site/_ro/trn_rl_repo"):
    if _p not in sys.path:
        sys.path.insert(0, _p)

import numpy as np

import concourse.bass as bass
import concourse.mybir as mybir
from concourse import bacc, tile
from concourse.bass_utils import run_bass_kernel_spmd

f32 = mybir.dt.float32
bf16 = mybir.dt.bfloat16
f8 = mybir.dt.float8e4
DR = mybir.MatmulPerfMode.DoubleRow
P = 128
NK2 = 4                      # DoubleRow k-pairs (each covers 256 of D)
WSCALE = 32.0                # W/w_copy pre-scale into fp8 range
INV = 1.0 / WSCALE

B, T, S, C, V, D = 16, 128, 512, 512, 50000, 1024
BT = B * T
NCORES = 8
VSH = V // NCORES            # 6250 vocab columns per core
VSHP = 6272                  # padded to 49*128 (pad cols get b=-1e30 -> exp=0)
NK = D // P                  # 8 contraction k-tiles
NT = BT // P                 # 16 token tiles
NS = S // P                  # 4 copy-branch contraction k-tiles
BSH = B // NCORES            # 2 batches per core (copy branch)
GROUPS = [4, 4, 4, 3, 1]     # token tiles per normalizer AllReduce
WSPL = 3072                  # W column-split point (per-k DMA granularity)
# vocab sweep: PSUM groups of 6x512-wide tiles (6 banks), then the tail
VGS = [(0, [512] * 6),
       (3072, [512] * 6),
       (6144, [128])]
NVT = sum(len(s) for _, s in VGS)     # 13 accum columns


def build_nc():
    nc = bacc.Bacc(
        "TRN2", target_bir_lowering=False, debug=False, num_devices=NCORES
    )
    # [tt][din][k*128+t] = hidden[tt*128+t, k*128+din]
    hT_d = nc.declare_dram_parameter("hT", [NT, P, D], f8, isOutput=False)
    # [k][din][v] = W_shard[v, k*128+din]
    wT_d = nc.declare_dram_parameter("wT", [NK2, P, 2, VSHP], f8, isOutput=False)
    b_d = nc.declare_dram_parameter("b_row", [1, VSHP], bf16, isOutput=False)
    wc_d = nc.declare_dram_parameter("w_copyT", [P, NK2, 2], f8, isOutput=False)
    bc_d = nc.declare_dram_parameter("b_copy", [1, 1], bf16, isOutput=False)
    # [ks][s][t] = attn_shard[t, ks*128+s]
    at_d = nc.declare_dram_parameter("attnT", [NS, P, BSH * T], bf16, isOutput=False)
    # [i*NS+ks][s][c] = src_map[i, ks*128+s, c]
    src_d = nc.declare_dram_parameter("srcm", [BSH * NS, P, C], bf16, isOutput=False)
    hcb_d = nc.declare_dram_parameter("hidden_cb", [BSH, P, D], bf16, isOutput=False)
    wc16_d = nc.declare_dram_parameter("w_copyT16", [P, NK], bf16, isOutput=False)
    out_p = nc.declare_dram_parameter("out_prob", [BT, VSHP], bf16, isOutput=True)
    out_c = nc.declare_dram_parameter("copy_prob", [BSH * T, C], f32, isOutput=True)

    Exp = mybir.ActivationFunctionType.Exp
    add = mybir.AluOpType.add
    mult = mybir.AluOpType.mult

    with tile.TileContext(nc, num_cores=NCORES) as tc:
        from contextlib import ExitStack

        with ExitStack() as stack:
            constp = stack.enter_context(tc.tile_pool(name="const", bufs=1))
            wpool = stack.enter_context(tc.tile_pool(name="wres", bufs=1))
            htp = stack.enter_context(tc.tile_pool(name="hT", bufs=3))
            sumsp = stack.enter_context(tc.tile_pool(name="sums", bufs=3))
            smallp = stack.enter_context(tc.tile_pool(name="small", bufs=8))
            lsgp = stack.enter_context(tc.tile_pool(name="lsg", bufs=4))
            cbp = stack.enter_context(tc.tile_pool(name="cb", bufs=1))
            psmm = stack.enter_context(
                tc.tile_pool(name="psum_mm", bufs=6, space="PSUM"))
            pssm = stack.enter_context(
                tc.tile_pool(name="psum_sm", bufs=1, space="PSUM"))
            pscb = stack.enter_context(
                tc.tile_pool(name="psum_cb", bufs=1, space="PSUM"))
            dramp = stack.enter_context(
                tc.tile_pool(name="ccdram", bufs=2 * len(GROUPS), space="DRAM"))

            # ---- tiny constants ----
            ones1 = constp.tile([1, P], bf16)
            nc.gpsimd.memset(ones1[:, :], 1.0)
            wcT = constp.tile([P, NK2, 2], f8)
            nc.sync.dma_start(wcT[:, :, :], wc_d.ap())
            wcT16 = constp.tile([P, NK], bf16)
            nc.sync.dma_start(wcT16[:, :], wc16_d.ap())
            bcT = constp.tile([1, 1], bf16)
            nc.sync.dma_start(bcT[:, :], bc_d.ap())
            bc_ps = pssm.tile([P, 1], f32, tag="pc", bufs=1)
            nc.tensor.matmul(bc_ps[:, :], ones1[0:1, :], bcT[0:1, :],
                             start=True, stop=True)
            bcNeg = constp.tile([P, 1], f32)
            nc.vector.tensor_scalar(bcNeg[:, :], bc_ps[:, :], -1.0, None, mult)

            pcall = constp.tile([P, NT], f32)
            S_all = constp.tile([P, NT], f32)
            browp = stack.enter_context(tc.tile_pool(name="brow", bufs=1))
            b_row = browp.tile([1, VSHP], bf16)
            nc.sync.dma_start(b_row[:, :], b_d.ap())

            # ---- copy-branch input DMAs issued early; compute at the end
            attnT = cbp.tile([P, NS, BSH * T], bf16)
            for ks in range(NS):
                nc.sync.dma_start(attnT[:, ks, :], at_d.ap()[ks])
            hcb_t, src_t = [], []
            for i in range(BSH):
                hcb = cbp.tile([P, D], bf16, name=f"hcb{i}")
                nc.sync.dma_start(hcb[:, :], hcb_d.ap()[i])
                hcb_t.append(hcb)
                srcT = cbp.tile([P, NS, C], bf16, name=f"src{i}")
                for ks in range(NS):
                    nc.sync.dma_start(srcT[:, ks, :], src_d.ap()[i * NS + ks])
                src_t.append(srcT)

            # ---- prefetch first hidden tiles, then stream W (biggest) ----
            hT_t = {}
            for tt in range(min(3, NT)):
                t_ = htp.tile([P, NK2, 2, P], f8, tag="hT")
                nc.sync.dma_start(t_[:, :, :, :], hT_d.ap()[tt])
                hT_t[tt] = t_
            # W split in column halves per k so the first sweep group's
            # matmuls only wait on the first 6.3MB of W, not all 12.6MB.
            w_a, w_b = [], []
            for k in range(NK2):
                wa = wpool.tile([P, 2, WSPL], f8, name=f"wa{k}")
                nc.sync.dma_start(wa[:, :, :], wT_d.ap()[k, :, :, 0:WSPL])
                w_a.append(wa)
            for k in range(NK2):
                wb = wpool.tile([P, 2, VSHP - WSPL], f8, name=f"wb{k}")
                nc.sync.dma_start(wb[:, :, :], wT_d.ap()[k, :, :, WSPL:VSHP])
                w_b.append(wb)

            def w_slice(k, c0, w):
                if c0 + w <= WSPL:
                    return w_a[k][:, :, c0:c0 + w]
                assert c0 >= WSPL
                return w_b[k][:, :, c0 - WSPL:c0 - WSPL + w]

            # ---- bias broadcast b_bc[P, VSHP] (pad cols already -1e30) ----
            b_bc = wpool.tile([P, VSHP], bf16, name="b_bc")
            for c0 in range(0, VSHP, 512):
                w = min(512, VSHP - c0)
                pm = psmm.tile([P, 512], f32, tag="mm")
                nc.tensor.matmul(pm[:, :w], ones1[0:1, :],
                                 b_row[0:1, c0:c0 + w],
                                 start=True, stop=True)
                nc.vector.tensor_copy(out=b_bc[:, c0:c0 + w], in_=pm[:, :w])

            # exp stays in SBUF: up to 4 token tiles in flight
            expp = stack.enter_context(tc.tile_pool(name="exp", bufs=8))
            outsp = stack.enter_context(tc.tile_pool(name="outst", bufs=2))

            # ---- main loop ----
            def phase_a(tt, lsg, j):
                hT = hT_t.pop(tt)
                exp_t = expp.tile([P, VSHP], bf16, tag="exp")
                sums = sumsp.tile([P, NVT], f32, tag="sums")
                pps = pssm.tile([P, 1], f32, tag="pc", bufs=1)
                vt = 0
                for gi, (g0, widths) in enumerate(VGS):
                    slices = []
                    c0 = g0
                    for w in widths:
                        pm = psmm.tile([P, 512], f32, tag="mm")
                        slices.append((c0, w, pm))
                        c0 += w
                    # hold hT[q] stationary across all banks of this sweep
                    for k in range(NK2):
                        lhsT = hT[:, k, :, :]
                        for (c0, w, pm) in slices:
                            if w >= 512:
                                nc.tensor.matmul(
                                    pm[:, :w], lhsT, w_slice(k, c0, w),
                                    start=(k == 0), stop=(k == NK2 - 1),
                                    perf_mode=DR)
                            else:
                                ws = w_slice(k, c0, w)
                                for i in range(2):
                                    nc.tensor.matmul(
                                        pm[:, :w], hT[:, k, i, :],
                                        ws[:, i, :],
                                        start=(k == 0 and i == 0),
                                        stop=(k == NK2 - 1 and i == 1))
                        if gi == 0:
                            for i in range(2):
                                nc.tensor.matmul(
                                    pps[:, :], hT[:, k, i, :],
                                    wcT[:, k, i:i + 1],
                                    start=(k == 0 and i == 0),
                                    stop=(k == NK2 - 1 and i == 1))
                    for (c0, w, pm) in slices:
                        nc.vector.tensor_tensor(
                            pm[:, :w], pm[:, :w], b_bc[:, c0:c0 + w], add)
                        nc.scalar.activation(
                            exp_t[:, c0:c0 + w], pm[:, :w], Exp, scale=INV,
                            accum_out=sums[:, vt:vt + 1])
                        vt += 1
                nc.scalar.activation(pcall[:, tt:tt + 1], pps[:, :], Exp,
                                     bias=bcNeg[:, :], scale=-INV)
                nc.vector.tensor_reduce(lsg[:, j:j + 1], sums[:, :NVT],
                                        mybir.AxisListType.X, add)
                nxt = tt + 3
                if nxt < NT:
                    t_ = htp.tile([P, NK2, 2, P], f8, tag="hT")
                    nc.sync.dma_start(t_[:, :, :, :], hT_d.ap()[nxt])
                    hT_t[nxt] = t_
                return exp_t

            def phase_c(tt, exp_t):
                y = pcall[:, tt:tt + 1]
                t1 = smallp.tile([P, 1], f32, tag="sc")
                nc.vector.tensor_scalar(t1[:, :], y, 1.0, None, add)
                t2 = smallp.tile([P, 1], f32, tag="sc")
                nc.vector.tensor_tensor(t2[:, :], t1[:, :],
                                        S_all[:, tt:tt + 1], mult)
                t3 = smallp.tile([P, 1], f32, tag="sc")
                nc.vector.reciprocal(t3[:, :], t2[:, :])
                rs = smallp.tile([P, 1], f32, tag="sc")
                nc.vector.tensor_tensor(rs[:, :], t3[:, :], y, mult)
                for c0 in range(0, VSHP, 2048):
                    w = min(2048, VSHP - c0)
                    ost = outsp.tile([P, 2048], bf16, tag="outst")
                    nc.vector.tensor_scalar(ost[:, :w], exp_t[:, c0:c0 + w],
                                            rs[:, :], None, mult)
                    nc.sync.dma_start(
                        out_p.ap()[tt * P:(tt + 1) * P, c0:c0 + w],
                        ost[:, :w])

            def copy_branch():
                for i in range(BSH):
                    pps = pssm.tile([P, 1], f32, tag="pc", bufs=1)
                    for k in range(NK):
                        nc.tensor.matmul(
                            pps[:, :], hcb_t[i][:, k * P:(k + 1) * P],
                            wcT16[:, k:k + 1],
                            start=(k == 0), stop=(k == NK - 1))
                    ycb = smallp.tile([P, 1], f32, tag="sc")
                    nc.scalar.activation(ycb[:, :], pps[:, :], Exp,
                                         bias=bcNeg[:, :], scale=-1.0)
                    t1 = smallp.tile([P, 1], f32, tag="sc")
                    nc.vector.tensor_scalar(t1[:, :], ycb[:, :], 1.0, None, add)
                    pcb = smallp.tile([P, 1], f32, tag="sc")
                    nc.vector.reciprocal(pcb[:, :], t1[:, :])
                    cps = pscb.tile([P, C], f32, tag="cb", bufs=1)
                    for ks in range(NS):
                        nc.tensor.matmul(
                            cps[:, :], attnT[:, ks, i * P:(i + 1) * P],
                            src_t[i][:, ks, :],
                            start=(ks == 0), stop=(ks == NS - 1))
                    cstg = cbp.tile([P, C], f32, name=f"cst{i}")
                    nc.vector.tensor_scalar(cstg[:, :], cps[:, :], pcb[:, :],
                                            None, mult)
                    nc.sync.dma_start(out_c.ap()[i * P:(i + 1) * P, :],
                                      cstg[:, :])

            exp_tiles = {}
            groups = []
            t0 = 0
            for gsz in GROUPS:
                groups.append(list(range(t0, t0 + gsz)))
                t0 += gsz
            assert t0 == NT

            for g, grp in enumerate(groups):
                lsg = lsgp.tile([P, len(grp)], f32, tag="lsg")
                for j, tt in enumerate(grp):
                    exp_tiles[tt] = phase_a(tt, lsg, j)
                cc_in = dramp.tile([P, len(grp)], f32, tag="cc_in")
                cc_out = dramp.tile([P, len(grp)], f32, tag="cc_out")
                nc.sync.dma_start(cc_in[:, :], lsg[:, :])
                nc.gpsimd.collective_compute(
                    "AllReduce", add,
                    replica_groups=[list(range(NCORES))],
                    ins=[cc_in.opt()], outs=[cc_out.opt()],
                )
                nc.sync.dma_start(
                    S_all[:, grp[0]:grp[0] + len(grp)], cc_out[:, :])
                if g == len(groups) - 1:
                    # fill the final AllReduce's latency shadow
                    copy_branch()
                if g >= 1:
                    for tt in groups[g - 1]:
                        phase_c(tt, exp_tiles.pop(tt))
            for tt in groups[-1]:
                phase_c(tt, exp_tiles.pop(tt))

    nc.finalize()
    return nc


_CACHE = {}


def _get_nc():
    if "nc" not in _CACHE:
        _CACHE["nc"] = build_nc()
    return _CACHE["nc"]


def make_in_maps(hidden, attn, src_map, W, b, w_copy, b_copy, pad_idx):
    import ml_dtypes

    bF = ml_dtypes.bfloat16
    hidden = np.asarray(hidden, np.float32)
    attn = np.asarray(attn, np.float32)
    src_map = np.asarray(src_map, np.float32)
    W = np.asarray(W, np.float32)
    b = np.asarray(b, np.float32)
    w_copy = np.asarray(w_copy, np.float32)
    b_copy = np.asarray(b_copy, np.float32)
    pad = int(np.asarray(pad_idx))

    f8np = ml_dtypes.float8_e4m3fn
    # hidden^T DoubleRow tiles: [tt, din, q, i, t]
    H3 = hidden.reshape(NT, P, NK2, 2, P).transpose(0, 4, 2, 3, 1)
    H3 = np.ascontiguousarray(H3.reshape(NT, P, D).astype(f8np))
    wcT = np.ascontiguousarray(
        (w_copy * WSCALE).reshape(NK2, 2, P).transpose(2, 0, 1).astype(f8np))
    wcT16 = np.ascontiguousarray(w_copy.reshape(NK, P).T.astype(bF))
    H16 = hidden.reshape(NT, P, NK, P).transpose(0, 3, 2, 1)
    H16 = np.ascontiguousarray(H16.reshape(NT, P, D).astype(bF))
    bc = np.ascontiguousarray(b_copy.reshape(1, 1).astype(bF))

    in_maps = []
    for c in range(NCORES):
        lo, hi = c * VSH, (c + 1) * VSH
        Wp = np.zeros((VSHP, D), np.float32)
        Wp[:VSH] = W[lo:hi] * WSCALE
        wT = np.ascontiguousarray(
            Wp.reshape(VSHP, NK2, 2, P).transpose(1, 3, 2, 0).astype(f8np))
        bsl = np.full((VSHP,), -1e30, np.float32)
        bsl[:VSH] = b[lo:hi] * WSCALE
        if lo <= pad < hi:
            bsl[pad - lo] = -1e30
        b_rowA = np.ascontiguousarray(bsl.reshape(1, VSHP).astype(bF))
        a_sl = attn[c * BSH * T:(c + 1) * BSH * T]
        attnT = np.ascontiguousarray(
            a_sl.reshape(BSH * T, NS, P).transpose(1, 2, 0).astype(bF))
        s_sl = src_map[c * BSH:(c + 1) * BSH]
        srcm = np.ascontiguousarray(
            s_sl.reshape(BSH * NS, P, C).astype(bF))
        in_maps.append({
            "hT": H3,
            "wT": wT,
            "b_row": b_rowA,
            "w_copyT": wcT,
            "b_copy": bc,
            "attnT": attnT,
            "srcm": srcm,
            "hidden_cb": np.ascontiguousarray(H16[c * BSH:(c + 1) * BSH]),
            "w_copyT16": wcT16,
        })
    return in_maps


def assemble(results):
    out_prob = np.concatenate(
        [np.asarray(r["out_prob"], np.float32)[:, :VSH] for r in results],
        axis=1)
    copy_prob = np.concatenate(
        [np.asarray(r["copy_prob"], np.float32) for r in results], axis=0)
    return np.concatenate([out_prob, copy_prob], axis=1)


FULL_CFG = dict(B=B, T=T, S=S, C=C, V=V, D=D)


def run(cfg, inputs, trace=False):
    """test.py interface: run(K.FULL_CFG, np_inputs, trace=True)."""
    nc = _get_nc()
    in_maps = make_in_maps(**inputs)
    res = run_bass_kernel_spmd(nc, in_maps, list(range(NCORES)), trace=trace)
    return assemble(res.results), res


def kernel(**inputs) -> np.ndarray:
    out, _ = run(FULL_CFG, inputs, trace=False)
    return out

